# revision 1
# baseline (speedup 1.0000x reference)
"""Bilinear RoI pooling (7x7) on 8 Trainium2 NeuronCores.

Strategy (data-parallel over RoIs, per the sharding hint):
  - B=1024 boxes split into 8 slices of 128; the feature map is replicated.
  - Host pads features (128,128,512) -> (130,132,512) with a zero border
    (1 row/col top/left, 2 pad cols left+right) and corner indices are
    clamped so every out-of-bounds bilinear corner reads a zero row instead
    of needing an in-bounds mask.
  - Per core, per (box, grid-point): indirect-DMA gathers of 1024
    contiguous f32 (the x0/x0+1 row pair for each of the two y corners),
    then a 4-term per-partition weighted sum on the vector engine.
  - Gathers are grouped: one indirect DMA fetches gk grid points' worth of
    A- and B-rows (2*gk indices/partition) to amortize SWDGE overhead.

Device layout: partition = box (128/core); 49 grid points along free dim.
"""

import numpy as np

P = 128          # boxes per core == SBUF partitions
C = 512          # channels
NPT = 49         # 7*7 grid points
HP, WP = 130, 132
NROW = HP * WP   # 17160 padded rows of C floats
NCORES = 8
MAGIC = 12582912.0  # 1.5*2^23: x+MAGIC stays in [2^23,2^24) where ulp == 1

_STATE = {}


# NOTE: multi-index offset APs (merge_ab / gk>1) pass CoreSim but produce
# garbage on real hardware — the HW indirect DMA only honors [P,1] offsets.
def _build_nc(repeats=1, skip_b=False, skip_blend=False, bufs=16, gk=1,
              gbufs=2, abufs=3, merge_ab=False, act_offload=False,
              store_group=7):
    import concourse.bass as bass
    import concourse.bacc as bacc
    import concourse.tile as tile
    from concourse import mybir

    F32 = mybir.dt.float32
    I32 = mybir.dt.int32
    Alu = mybir.AluOpType

    nc = bacc.Bacc()
    fpad = nc.declare_dram_parameter("fpad", [NROW, C], F32, isOutput=False)
    boxes = nc.declare_dram_parameter("boxes", [P, 4], F32, isOutput=False)
    grid = nc.declare_dram_parameter("grid", [P, 2 * NPT], F32, isOutput=False)
    out = nc.declare_dram_parameter("out", [P, NPT * C], F32, isOutput=True)

    with tile.TileContext(nc) as tc:
        with (
            tc.tile_pool(name="const", bufs=1) as cpool,
            tc.tile_pool(name="gpool", bufs=gbufs) as gpool,
            tc.tile_pool(name="apool", bufs=abufs) as apool,
            tc.tile_pool(name="work", bufs=bufs) as wpool,
        ):
            bx = cpool.tile([P, 4], F32)
            nc.sync.dma_start(out=bx[:], in_=boxes[:])
            g = cpool.tile([P, 2 * NPT], F32)
            nc.sync.dma_start(out=g[:], in_=grid[:])
            BY = g[:, 0:NPT]
            BX = g[:, NPT:2 * NPT]

            xc, yc = bx[:, 0:1], bx[:, 1:2]
            bw, bh = bx[:, 2:3], bx[:, 3:4]

            # per-box scale/translate: yf = BY*(0.5*bh-0.5) + (yc-1)
            sy = cpool.tile([P, 1], F32)
            nc.vector.tensor_scalar(out=sy[:], in0=bh, scalar1=0.5, scalar2=-0.5,
                                    op0=Alu.mult, op1=Alu.add)
            sx = cpool.tile([P, 1], F32)
            nc.vector.tensor_scalar(out=sx[:], in0=bw, scalar1=0.5, scalar2=-0.5,
                                    op0=Alu.mult, op1=Alu.add)
            ty = cpool.tile([P, 1], F32)
            nc.vector.tensor_scalar(out=ty[:], in0=yc, scalar1=-1.0, scalar2=None,
                                    op0=Alu.add)
            tx = cpool.tile([P, 1], F32)
            nc.vector.tensor_scalar(out=tx[:], in0=xc, scalar1=-1.0, scalar2=None,
                                    op0=Alu.add)

            yf = cpool.tile([P, NPT], F32)
            nc.vector.tensor_scalar(out=yf[:], in0=BY, scalar1=sy[:, 0:1],
                                    scalar2=ty[:, 0:1], op0=Alu.mult, op1=Alu.add)
            xf = cpool.tile([P, NPT], F32)
            nc.vector.tensor_scalar(out=xf[:], in0=BX, scalar1=sx[:, 0:1],
                                    scalar2=tx[:, 0:1], op0=Alu.mult, op1=Alu.add)

            def floor_frac(src, nm):
                r = cpool.tile([P, NPT], F32, tag=f"r{nm}")
                nc.vector.tensor_scalar(out=r[:], in0=src[:], scalar1=MAGIC,
                                        scalar2=-MAGIC, op0=Alu.add, op1=Alu.add)
                m = cpool.tile([P, NPT], F32, tag=f"m{nm}")
                nc.vector.tensor_tensor(out=m[:], in0=r[:], in1=src[:], op=Alu.is_gt)
                fl = cpool.tile([P, NPT], F32, tag=f"f{nm}")
                nc.vector.tensor_tensor(out=fl[:], in0=r[:], in1=m[:], op=Alu.subtract)
                fr = cpool.tile([P, NPT], F32, tag=f"w{nm}")
                nc.vector.tensor_tensor(out=fr[:], in0=src[:], in1=fl[:], op=Alu.subtract)
                return fl, fr

            y0, wy = floor_frac(yf[:], "y")
            x0, wx = floor_frac(xf[:], "x")

            wyc = cpool.tile([P, NPT], F32)
            nc.vector.tensor_scalar(out=wyc[:], in0=wy[:], scalar1=-1.0, scalar2=1.0,
                                    op0=Alu.mult, op1=Alu.add)
            wxc = cpool.tile([P, NPT], F32)
            nc.vector.tensor_scalar(out=wxc[:], in0=wx[:], scalar1=-1.0, scalar2=1.0,
                                    op0=Alu.mult, op1=Alu.add)

            wA0 = cpool.tile([P, NPT], F32)
            nc.vector.tensor_tensor(out=wA0[:], in0=wyc[:], in1=wxc[:], op=Alu.mult)
            wA1 = cpool.tile([P, NPT], F32)
            nc.vector.tensor_tensor(out=wA1[:], in0=wyc[:], in1=wx[:], op=Alu.mult)
            wB0 = cpool.tile([P, NPT], F32)
            nc.vector.tensor_tensor(out=wB0[:], in0=wy[:], in1=wxc[:], op=Alu.mult)
            wB1 = cpool.tile([P, NPT], F32)
            nc.vector.tensor_tensor(out=wB1[:], in0=wy[:], in1=wx[:], op=Alu.mult)

            # gather row indices into the padded (130,132) map, in 512-elem
            # units:
            #   idxA = (clamp(y0,-1,128)+1)*132 + clamp(x0,-2,128)+2
            #   idxB = (clamp(y0,-2,127)+2)*132 + clamp(x0,-2,128)+2
            cy0 = cpool.tile([P, NPT], F32)
            nc.vector.tensor_scalar(out=cy0[:], in0=y0[:], scalar1=-1.0, scalar2=128.0,
                                    op0=Alu.max, op1=Alu.min)
            cy1 = cpool.tile([P, NPT], F32)
            nc.vector.tensor_scalar(out=cy1[:], in0=y0[:], scalar1=127.0, scalar2=-2.0,
                                    op0=Alu.min, op1=Alu.max)
            cxc = cpool.tile([P, NPT], F32)
            nc.vector.tensor_scalar(out=cxc[:], in0=x0[:], scalar1=-2.0, scalar2=128.0,
                                    op0=Alu.max, op1=Alu.min)

            affA = cpool.tile([P, NPT], F32)
            nc.vector.tensor_scalar(out=affA[:], in0=cy0[:], scalar1=float(WP),
                                    scalar2=float(WP + 2), op0=Alu.mult, op1=Alu.add)
            nc.vector.tensor_tensor(out=affA[:], in0=affA[:], in1=cxc[:], op=Alu.add)
            affB = cpool.tile([P, NPT], F32)
            nc.vector.tensor_scalar(out=affB[:], in0=cy1[:], scalar1=float(WP),
                                    scalar2=float(2 * WP + 2), op0=Alu.mult, op1=Alu.add)
            nc.vector.tensor_tensor(out=affB[:], in0=affB[:], in1=cxc[:], op=Alu.add)

            # NOTE: do NOT try to skip pad-zero gathers via
            # bounds_check+oob_is_err=False — a skipped descriptor leaves the
            # dest SBUF unwritten, and 0 * stale-NaN = NaN when uninitialized
            # SBUF holds NaN bit patterns (observed nondeterministically on
            # HW). The zero-padded feature map already makes out-of-bounds
            # corners contribute exactly 0.

            import concourse.bass as _b

            if gk > 1:
                # grouped gathers: one indirect DMA per gk grid points, with
                # A indices then B indices per group:
                #   idxAB cols [g*2gk : g*2gk+gk]       = idxA[t0 : t0+gk]
                #   idxAB cols [g*2gk+gk : (g+1)*2gk]   = idxB[t0 : t0+gk]
                ng = NPT // gk
                assert NPT % gk == 0
                idxAB = cpool.tile([P, 2 * NPT], I32)
                idxAB4 = idxAB[:].rearrange("p (g two k) -> p g two k",
                                            g=ng, two=2, k=gk)
                nc.vector.tensor_copy(
                    out=idxAB4[:, :, 0, :],
                    in_=affA[:].rearrange("p (g k) -> p g k", g=ng, k=gk))
                nc.vector.tensor_copy(
                    out=idxAB4[:, :, 1, :],
                    in_=affB[:].rearrange("p (g k) -> p g k", g=ng, k=gk))
                for rep in range(repeats):
                    for g_i in range(ng):
                        gfat = gpool.tile([P, gk * 4 * C], F32, tag="gfat")
                        nc.gpsimd.indirect_dma_start(
                            out=gfat[:], out_offset=None, in_=fpad[:],
                            in_offset=_b.IndirectOffsetOnAxis(
                                ap=idxAB[:, g_i * 2 * gk:(g_i + 1) * 2 * gk],
                                axis=0),
                        )
                        afat = apool.tile([P, gk * C], F32, tag="afat")
                        for k in range(gk):
                            t = g_i * gk + k
                            a0 = gfat[:, k * 2 * C: k * 2 * C + C]
                            a1 = gfat[:, k * 2 * C + C: (k + 1) * 2 * C]
                            b0 = gfat[:, (gk + k) * 2 * C: (gk + k) * 2 * C + C]
                            b1 = gfat[:, (gk + k) * 2 * C + C: (gk + k + 1) * 2 * C]
                            ac = afat[:, k * C:(k + 1) * C]
                            nc.vector.tensor_scalar(
                                out=ac, in0=a0, scalar1=wA0[:, t:t + 1],
                                scalar2=None, op0=Alu.mult)
                            nc.vector.scalar_tensor_tensor(
                                out=ac, in0=a1, scalar=wA1[:, t:t + 1], in1=ac,
                                op0=Alu.mult, op1=Alu.add)
                            nc.vector.scalar_tensor_tensor(
                                out=ac, in0=b0, scalar=wB0[:, t:t + 1], in1=ac,
                                op0=Alu.mult, op1=Alu.add)
                            nc.vector.scalar_tensor_tensor(
                                out=ac, in0=b1, scalar=wB1[:, t:t + 1], in1=ac,
                                op0=Alu.mult, op1=Alu.add)
                        nc.sync.dma_start(
                            out=out[:, g_i * gk * C:(g_i + 1) * gk * C],
                            in_=afat[:])
            elif merge_ab:
                # one gather per point with both y-corner indices:
                # idxAB2 cols [2t, 2t+1] = idxA[t], idxB[t]
                idxAB2 = cpool.tile([P, 2 * NPT], I32)
                iv = idxAB2[:].rearrange("p (t two) -> p t two", t=NPT, two=2)
                nc.vector.tensor_copy(out=iv[:, :, 0], in_=affA[:])
                nc.vector.tensor_copy(out=iv[:, :, 1], in_=affB[:])
                for t in [t for _ in range(repeats) for t in range(NPT)]:
                    gAB = wpool.tile([P, 4 * C], F32, tag="gAB")
                    nc.gpsimd.indirect_dma_start(
                        out=gAB[:], out_offset=None, in_=fpad[:],
                        in_offset=_b.IndirectOffsetOnAxis(
                            ap=idxAB2[:, 2 * t:2 * t + 2], axis=0),
                    )
                    acc = wpool.tile([P, C], F32, tag="acc")
                    if act_offload:
                        import concourse.mybir as _mb
                        m = wpool.tile([P, C], F32, tag="actm")
                        nc.scalar.activation(out=m[:], in_=gAB[:, 2 * C:3 * C],
                                             func=_mb.ActivationFunctionType.Copy,
                                             scale=wB0[:, t:t + 1])
                        nc.vector.scalar_tensor_tensor(out=acc[:],
                                                       in0=gAB[:, 0:C],
                                                       scalar=wA0[:, t:t + 1],
                                                       in1=m[:],
                                                       op0=Alu.mult, op1=Alu.add)
                    else:
                        nc.vector.tensor_scalar(out=acc[:], in0=gAB[:, 0:C],
                                                scalar1=wA0[:, t:t + 1],
                                                scalar2=None, op0=Alu.mult)
                    nc.vector.scalar_tensor_tensor(out=acc[:], in0=gAB[:, C:2 * C],
                                                   scalar=wA1[:, t:t + 1],
                                                   in1=acc[:],
                                                   op0=Alu.mult, op1=Alu.add)
                    if not act_offload:
                        nc.vector.scalar_tensor_tensor(out=acc[:],
                                                       in0=gAB[:, 2 * C:3 * C],
                                                       scalar=wB0[:, t:t + 1],
                                                       in1=acc[:],
                                                       op0=Alu.mult, op1=Alu.add)
                    nc.vector.scalar_tensor_tensor(out=acc[:],
                                                   in0=gAB[:, 3 * C:4 * C],
                                                   scalar=wB1[:, t:t + 1],
                                                   in1=acc[:],
                                                   op0=Alu.mult, op1=Alu.add)
                    nc.sync.dma_start(out=out[:, t * C:(t + 1) * C], in_=acc[:])
            elif store_group > 1:
                # same per-point gathers/blends, but blends write into a
                # [P, store_group*C] tile flushed by one contiguous store
                # per group (fewer, larger store descriptors)
                sg = store_group
                assert NPT % sg == 0
                idxA = cpool.tile([P, NPT], I32)
                nc.vector.tensor_copy(out=idxA[:], in_=affA[:])
                idxB = cpool.tile([P, NPT], I32)
                nc.vector.tensor_copy(out=idxB[:], in_=affB[:])
                for rep in range(repeats):
                    for g_i in range(NPT // sg):
                        afat = apool.tile([P, sg * C], F32, tag="afat")
                        for k in range(sg):
                            t = g_i * sg + k
                            gA = wpool.tile([P, 2 * C], F32, tag="gA")
                            nc.gpsimd.indirect_dma_start(
                                out=gA[:], out_offset=None, in_=fpad[:],
                                in_offset=_b.IndirectOffsetOnAxis(
                                    ap=idxA[:, t:t + 1], axis=0))
                            gB = wpool.tile([P, 2 * C], F32, tag="gB")
                            nc.gpsimd.indirect_dma_start(
                                out=gB[:], out_offset=None, in_=fpad[:],
                                in_offset=_b.IndirectOffsetOnAxis(
                                    ap=idxB[:, t:t + 1], axis=0))
                            ac = afat[:, k * C:(k + 1) * C]
                            nc.vector.tensor_scalar(
                                out=ac, in0=gA[:, 0:C], scalar1=wA0[:, t:t + 1],
                                scalar2=None, op0=Alu.mult)
                            nc.vector.scalar_tensor_tensor(
                                out=ac, in0=gA[:, C:2 * C],
                                scalar=wA1[:, t:t + 1], in1=ac,
                                op0=Alu.mult, op1=Alu.add)
                            nc.vector.scalar_tensor_tensor(
                                out=ac, in0=gB[:, 0:C],
                                scalar=wB0[:, t:t + 1], in1=ac,
                                op0=Alu.mult, op1=Alu.add)
                            nc.vector.scalar_tensor_tensor(
                                out=ac, in0=gB[:, C:2 * C],
                                scalar=wB1[:, t:t + 1], in1=ac,
                                op0=Alu.mult, op1=Alu.add)
                        nc.sync.dma_start(
                            out=out[:, g_i * sg * C:(g_i + 1) * sg * C],
                            in_=afat[:])
            else:
                idxA = cpool.tile([P, NPT], I32)
                nc.vector.tensor_copy(out=idxA[:], in_=affA[:])
                idxB = cpool.tile([P, NPT], I32)
                nc.vector.tensor_copy(out=idxB[:], in_=affB[:])
                for t in [t for _ in range(repeats) for t in range(NPT)]:
                    gA = wpool.tile([P, 2 * C], F32, tag="gA")
                    nc.gpsimd.indirect_dma_start(
                        out=gA[:], out_offset=None, in_=fpad[:],
                        in_offset=_b.IndirectOffsetOnAxis(ap=idxA[:, t:t + 1],
                                                          axis=0),
                    )
                    if not skip_b:
                        gB = wpool.tile([P, 2 * C], F32, tag="gB")
                        nc.gpsimd.indirect_dma_start(
                            out=gB[:], out_offset=None, in_=fpad[:],
                            in_offset=_b.IndirectOffsetOnAxis(ap=idxB[:, t:t + 1],
                                                              axis=0),
                        )
                    if skip_blend:
                        nc.sync.dma_start(out=out[:, t * C:(t + 1) * C],
                                          in_=gA[:, 0:C])
                        continue
                    acc = wpool.tile([P, C], F32, tag="acc")
                    nc.vector.tensor_scalar(out=acc[:], in0=gA[:, 0:C],
                                            scalar1=wA0[:, t:t + 1], scalar2=None,
                                            op0=Alu.mult)
                    nc.vector.scalar_tensor_tensor(out=acc[:], in0=gA[:, C:2 * C],
                                                   scalar=wA1[:, t:t + 1],
                                                   in1=acc[:],
                                                   op0=Alu.mult, op1=Alu.add)
                    if not skip_b:
                        nc.vector.scalar_tensor_tensor(out=acc[:],
                                                       in0=gB[:, 0:C],
                                                       scalar=wB0[:, t:t + 1],
                                                       in1=acc[:],
                                                       op0=Alu.mult, op1=Alu.add)
                        nc.vector.scalar_tensor_tensor(out=acc[:],
                                                       in0=gB[:, C:2 * C],
                                                       scalar=wB1[:, t:t + 1],
                                                       in1=acc[:],
                                                       op0=Alu.mult, op1=Alu.add)
                    nc.sync.dma_start(out=out[:, t * C:(t + 1) * C], in_=acc[:])

    nc.compile()
    return nc


def _grid_const():
    base = np.linspace(-1.0, 1.0, 7).astype(np.float32)
    by = np.repeat(base, 7)
    bxx = np.tile(base, 7)
    g = np.concatenate([by, bxx])[None, :]
    return np.ascontiguousarray(np.broadcast_to(g, (P, 2 * NPT)).astype(np.float32))


def _pad_features(features):
    fp = np.zeros((HP, WP, C), dtype=np.float32)
    fp[1:129, 2:130, :] = features
    return fp.reshape(NROW, C)


def kernel(features, boxes, image_height=128, image_width=128):
    from concourse.bass_utils import run_bass_kernel_spmd

    if "nc" not in _STATE:
        _STATE["nc"] = _build_nc()
        _STATE["grid"] = _grid_const()
    nc = _STATE["nc"]

    fpad = _pad_features(np.asarray(features, dtype=np.float32))
    boxes = np.asarray(boxes, dtype=np.float32)
    gridc = _STATE["grid"]
    in_maps = [
        {
            "fpad": fpad,
            "boxes": np.ascontiguousarray(boxes[k * P:(k + 1) * P]),
            "grid": gridc,
        }
        for k in range(NCORES)
    ]
    res = run_bass_kernel_spmd(
        nc, in_maps, core_ids=list(range(NCORES)),
        trace=_STATE.get("trace", False),
    )
    _STATE["last"] = res
    out = np.concatenate(
        [res.results[k]["out"].reshape(P, 7, 7, C) for k in range(NCORES)], axis=0
    )
    return out



# revision 3
# speedup vs baseline: 2.0645x; 2.0645x over previous
"""Bilinear RoI pooling (7x7) on 8 Trainium2 NeuronCores.

Strategy (data-parallel over RoIs, per the sharding hint):
  - B=1024 boxes split into 8 slices of 128; the feature map is replicated.
  - Host builds a pair-interleaved fp16 copy of the zero-padded feature map:
    padded map P2 is (132,132,C) with a 2-px zero border; two row-pair copies
    E (rows 2e,2e+1) and O (rows 2o+1,2o+2) are stored as
    [pair, x, r, c] so the full 2x2 bilinear patch for any (y0,x0) corner is
    4*C CONTIGUOUS fp16 elements -> ONE 4KB gather descriptor per
    (box, grid point) instead of two, at half the f32 byte count.
  - Per core, per (box, grid-point): one indirect-DMA gather of 4*C fp16,
    then a 4-term per-partition weighted sum on the vector engine (fp16 data,
    fp16 per-partition scalar weights).
  - Out-of-bounds corners read zero border rows/cols (clamped indices), so
    no in-bounds masking is needed.

Device layout: partition = box (128/core); 49 grid points along free dim.
Output is fp16 on device; host casts to f32.
"""

import numpy as np

P = 128          # boxes per core == SBUF partitions
C = 512          # channels
NPT = 49         # 7*7 grid points
WP2 = 132        # padded width (2 zero cols each side)
HP2 = 132        # padded height (2 zero rows top, 2 bottom)
NBLK_E = 66      # even row-pairs (rows 0..131)
NBLK_O = 65      # odd row-pairs (rows 1..130)
NSLOT_E = NBLK_E * WP2
NSLOT = (NBLK_E + NBLK_O) * WP2   # 17292 slots of [2, C]
NCORES = 8
MAGIC = 12582912.0  # 1.5*2^23: x+MAGIC stays in [2^23,2^24) where ulp == 1

_STATE = {}


# NOTE: multi-index offset APs (merge_ab / gk>1 style) pass CoreSim but
# produce garbage on real hardware — the HW indirect DMA only honors [P,1]
# offsets. One gather per (box, grid point); the pair-interleaved layout
# makes that one gather cover all 4 bilinear corners.
def _build_nc(repeats=1, bufs=16, abufs=3, store_group=7, skip_blend=False,
              skip_gather=False):
    import concourse.bass as bass
    import concourse.bacc as bacc
    import concourse.tile as tile
    from concourse import mybir

    F32 = mybir.dt.float32
    F16 = mybir.dt.float16
    I32 = mybir.dt.int32
    Alu = mybir.AluOpType

    nc = bacc.Bacc()
    fgat = nc.declare_dram_parameter("fgat", [NSLOT, 2 * C], F16, isOutput=False)
    boxes = nc.declare_dram_parameter("boxes", [P, 4], F32, isOutput=False)
    grid = nc.declare_dram_parameter("grid", [P, 2 * NPT], F32, isOutput=False)
    out = nc.declare_dram_parameter("out", [P, NPT * C], F16, isOutput=True)

    with tile.TileContext(nc) as tc:
        with (
            tc.tile_pool(name="const", bufs=1) as cpool,
            tc.tile_pool(name="apool", bufs=abufs) as apool,
            tc.tile_pool(name="work", bufs=bufs) as wpool,
        ):
            bx = cpool.tile([P, 4], F32)
            nc.sync.dma_start(out=bx[:], in_=boxes[:])
            g = cpool.tile([P, 2 * NPT], F32)
            nc.sync.dma_start(out=g[:], in_=grid[:])
            BY = g[:, 0:NPT]
            BX = g[:, NPT:2 * NPT]

            xc, yc = bx[:, 0:1], bx[:, 1:2]
            bw, bh = bx[:, 2:3], bx[:, 3:4]

            # per-box scale/translate: yf = BY*(0.5*bh-0.5) + (yc-1)
            sy = cpool.tile([P, 1], F32)
            nc.vector.tensor_scalar(out=sy[:], in0=bh, scalar1=0.5, scalar2=-0.5,
                                    op0=Alu.mult, op1=Alu.add)
            sx = cpool.tile([P, 1], F32)
            nc.vector.tensor_scalar(out=sx[:], in0=bw, scalar1=0.5, scalar2=-0.5,
                                    op0=Alu.mult, op1=Alu.add)
            ty = cpool.tile([P, 1], F32)
            nc.vector.tensor_scalar(out=ty[:], in0=yc, scalar1=-1.0, scalar2=None,
                                    op0=Alu.add)
            tx = cpool.tile([P, 1], F32)
            nc.vector.tensor_scalar(out=tx[:], in0=xc, scalar1=-1.0, scalar2=None,
                                    op0=Alu.add)

            yf = cpool.tile([P, NPT], F32)
            nc.vector.tensor_scalar(out=yf[:], in0=BY, scalar1=sy[:, 0:1],
                                    scalar2=ty[:, 0:1], op0=Alu.mult, op1=Alu.add)
            xf = cpool.tile([P, NPT], F32)
            nc.vector.tensor_scalar(out=xf[:], in0=BX, scalar1=sx[:, 0:1],
                                    scalar2=tx[:, 0:1], op0=Alu.mult, op1=Alu.add)

            def floor_frac(src, nm):
                # round-to-nearest via the magic constant, then correct down
                r = cpool.tile([P, NPT], F32, tag=f"r{nm}")
                nc.vector.tensor_scalar(out=r[:], in0=src[:], scalar1=MAGIC,
                                        scalar2=-MAGIC, op0=Alu.add, op1=Alu.add)
                m = cpool.tile([P, NPT], F32, tag=f"m{nm}")
                nc.vector.tensor_tensor(out=m[:], in0=r[:], in1=src[:], op=Alu.is_gt)
                fl = cpool.tile([P, NPT], F32, tag=f"f{nm}")
                nc.vector.tensor_tensor(out=fl[:], in0=r[:], in1=m[:], op=Alu.subtract)
                fr = cpool.tile([P, NPT], F32, tag=f"w{nm}")
                nc.vector.tensor_tensor(out=fr[:], in0=src[:], in1=fl[:], op=Alu.subtract)
                return fl, fr

            y0, wy = floor_frac(yf[:], "y")
            x0, wx = floor_frac(xf[:], "x")

            wyc = cpool.tile([P, NPT], F32)
            nc.vector.tensor_scalar(out=wyc[:], in0=wy[:], scalar1=-1.0, scalar2=1.0,
                                    op0=Alu.mult, op1=Alu.add)
            wxc = cpool.tile([P, NPT], F32)
            nc.vector.tensor_scalar(out=wxc[:], in0=wx[:], scalar1=-1.0, scalar2=1.0,
                                    op0=Alu.mult, op1=Alu.add)

            # gathered layout per point: [A0, B0, A1, B1] = [(y0,x0),(y0+1,x0),
            # (y0,x0+1),(y0+1,x0+1)]; scalar-operand APs must stay f32
            w00 = cpool.tile([P, NPT], F32)   # A0: wyc*wxc
            nc.vector.tensor_tensor(out=w00[:], in0=wyc[:], in1=wxc[:], op=Alu.mult)
            w10 = cpool.tile([P, NPT], F32)   # B0: wy*wxc
            nc.vector.tensor_tensor(out=w10[:], in0=wy[:], in1=wxc[:], op=Alu.mult)
            w01 = cpool.tile([P, NPT], F32)   # A1: wyc*wx
            nc.vector.tensor_tensor(out=w01[:], in0=wyc[:], in1=wx[:], op=Alu.mult)
            w11 = cpool.tile([P, NPT], F32)   # B1: wy*wx
            nc.vector.tensor_tensor(out=w11[:], in0=wy[:], in1=wx[:], op=Alu.mult)

            # slot index into the pair-interleaved map:
            #   pyA  = clamp(y0+2, 0, 130); half = floor(pyA/2); par = pyA-2*half
            #   px   = clamp(x0, -2, 128) + 2
            #   slot = par*NSLOT_E + half*WP2 + px
            pyA = cpool.tile([P, NPT], F32)
            nc.vector.tensor_scalar(out=pyA[:], in0=y0[:], scalar1=2.0, scalar2=0.0,
                                    op0=Alu.add, op1=Alu.max)
            nc.vector.tensor_scalar(out=pyA[:], in0=pyA[:], scalar1=130.0,
                                    scalar2=None, op0=Alu.min)
            hf = cpool.tile([P, NPT], F32)
            nc.vector.tensor_scalar(out=hf[:], in0=pyA[:], scalar1=0.5,
                                    scalar2=None, op0=Alu.mult)
            half, _ = floor_frac(hf[:], "h")
            par = cpool.tile([P, NPT], F32)
            nc.vector.scalar_tensor_tensor(out=par[:], in0=half[:], scalar=-2.0,
                                           in1=pyA[:], op0=Alu.mult, op1=Alu.add)
            pxc = cpool.tile([P, NPT], F32)
            nc.vector.tensor_scalar(out=pxc[:], in0=x0[:], scalar1=-2.0, scalar2=128.0,
                                    op0=Alu.max, op1=Alu.min)
            slot = cpool.tile([P, NPT], F32)
            nc.vector.tensor_scalar(out=slot[:], in0=half[:], scalar1=float(WP2),
                                    scalar2=2.0, op0=Alu.mult, op1=Alu.add)
            nc.vector.tensor_tensor(out=slot[:], in0=slot[:], in1=pxc[:], op=Alu.add)
            nc.vector.scalar_tensor_tensor(out=slot[:], in0=par[:],
                                           scalar=float(NSLOT_E), in1=slot[:],
                                           op0=Alu.mult, op1=Alu.add)
            idx = cpool.tile([P, NPT], I32)
            nc.vector.tensor_copy(out=idx[:], in_=slot[:])

            import concourse.bass as _b

            sg = store_group
            assert NPT % sg == 0
            for rep in range(repeats):
                for g_i in range(NPT // sg):
                    afat = apool.tile([P, sg * C], F16, tag="afat")
                    for k in range(sg):
                        t = g_i * sg + k
                        g4 = wpool.tile([P, 4 * C], F16, tag="g4")
                        if not skip_gather:
                            nc.gpsimd.indirect_dma_start(
                                out=g4[:], out_offset=None, in_=fgat[:],
                                in_offset=_b.IndirectOffsetOnAxis(
                                    ap=idx[:, t:t + 1], axis=0))
                        ac = afat[:, k * C:(k + 1) * C]
                        if skip_blend:
                            nc.vector.tensor_copy(out=ac, in_=g4[:, 0:C])
                            continue
                        nc.vector.tensor_scalar(
                            out=ac, in0=g4[:, 0:C], scalar1=w00[:, t:t + 1],
                            scalar2=None, op0=Alu.mult)
                        nc.vector.scalar_tensor_tensor(
                            out=ac, in0=g4[:, C:2 * C], scalar=w10[:, t:t + 1],
                            in1=ac, op0=Alu.mult, op1=Alu.add)
                        nc.vector.scalar_tensor_tensor(
                            out=ac, in0=g4[:, 2 * C:3 * C], scalar=w01[:, t:t + 1],
                            in1=ac, op0=Alu.mult, op1=Alu.add)
                        nc.vector.scalar_tensor_tensor(
                            out=ac, in0=g4[:, 3 * C:4 * C], scalar=w11[:, t:t + 1],
                            in1=ac, op0=Alu.mult, op1=Alu.add)
                    nc.sync.dma_start(
                        out=out[:, g_i * sg * C:(g_i + 1) * sg * C],
                        in_=afat[:])

    nc.compile()
    return nc


def _grid_const():
    base = np.linspace(-1.0, 1.0, 7).astype(np.float32)
    by = np.repeat(base, 7)
    bxx = np.tile(base, 7)
    g = np.concatenate([by, bxx])[None, :]
    return np.ascontiguousarray(np.broadcast_to(g, (P, 2 * NPT)).astype(np.float32))


def _prep_fgat(features):
    """Pair-interleaved fp16 gather map: E (even row pairs) then O (odd)."""
    f = np.asarray(features, dtype=np.float32)
    p2 = np.zeros((HP2, WP2, C), dtype=np.float16)
    p2[2:130, 2:130, :] = f.astype(np.float16)
    # E[e, x, r, c] = p2[2e+r, x, c]; O[o, x, r, c] = p2[2o+1+r, x, c]
    e = np.ascontiguousarray(
        p2.reshape(NBLK_E, 2, WP2, C).transpose(0, 2, 1, 3)
    ).reshape(NSLOT_E, 2 * C)
    o = np.ascontiguousarray(
        p2[1:131].reshape(NBLK_O, 2, WP2, C).transpose(0, 2, 1, 3)
    ).reshape(NBLK_O * WP2, 2 * C)
    return np.concatenate([e, o], axis=0)


def _in_maps(features, boxes):
    fgat = _prep_fgat(features)
    boxes = np.asarray(boxes, dtype=np.float32)
    gridc = _grid_const()
    return [
        {
            "fgat": fgat,
            "boxes": np.ascontiguousarray(boxes[k * P:(k + 1) * P]),
            "grid": gridc,
        }
        for k in range(NCORES)
    ]


def kernel(features, boxes, image_height=128, image_width=128):
    from concourse.bass_utils import run_bass_kernel_spmd

    if "nc" not in _STATE:
        _STATE["nc"] = _build_nc()
    nc = _STATE["nc"]

    in_maps = _in_maps(features, boxes)
    res = run_bass_kernel_spmd(
        nc, in_maps, core_ids=list(range(NCORES)),
        trace=_STATE.get("trace", False),
    )
    _STATE["last"] = res
    out = np.concatenate(
        [res.results[k]["out"].reshape(P, 7, 7, C).astype(np.float32)
         for k in range(NCORES)],
        axis=0,
    )
    return out


# revision 13
# speedup vs baseline: 2.1611x; 1.0468x over previous
"""Bilinear RoI pooling (7x7) on 8 Trainium2 NeuronCores.

Strategy (data-parallel over RoIs, per the sharding hint):
  - B=1024 boxes split into 8 slices of 128; the feature map is replicated.
  - Host builds a pair-interleaved fp16 copy of the zero-padded feature map:
    padded map P2 is (132,132,C) with a 2-px zero border; two row-pair copies
    E (rows 2e,2e+1) and O (rows 2o+1,2o+2) are stored as [pair, x, r, c] so
    the full 2x2 bilinear patch for any (y0,x0) corner is 4*C CONTIGUOUS fp16
    elements -> ONE 4KB gather descriptor per (box, grid point).
  - Host also precomputes the per-(box,point) gather slot index and the four
    bilinear corner weights (O(B*49) scalar work, same spirit as the host-side
    feature padding; the O(B*49*C) gather+blend stays on device).
  - Per core, per (box, grid-point): one indirect-DMA gather of 4*C fp16,
    then a 4-term weighted sum split across the vector and scalar engines
    (2 muls on ACT, 2 muls + 3 adds on DVE) so neither engine exceeds the
    DMA roofline.
  - Out-of-bounds corners read zero border rows/cols (clamped indices), so no
    in-bounds masking is needed.

Device layout: partition = box (128/core); 49 grid points along free dim.
Output is fp16 on device; host casts to f32.
"""

import numpy as np

P = 128          # boxes per core == SBUF partitions
C = 512          # channels
NPT = 49         # 7*7 grid points
WP2 = 132        # padded width (2 zero cols each side)
HP2 = 132        # padded height (2 zero rows top, 2 bottom)
NBLK_E = 66      # even row-pairs (rows 0..131)
NBLK_O = 65      # odd row-pairs (rows 1..130)
NSLOT_E = NBLK_E * WP2
NSLOT = (NBLK_E + NBLK_O) * WP2   # 17292 slots of [2, C]
NCORES = 8

_STATE = {}


# NOTE: multi-index offset APs (merge_ab / gk>1 style) pass CoreSim but
# produce garbage on real hardware — the HW indirect DMA only honors [P,1]
# offsets. One gather per (box, grid point); the pair-interleaved layout
# makes that one gather cover all 4 bilinear corners.
def _build_nc(repeats=1, bufs=12, abufs=3, tbufs=6, store_group=7,
              mode="actsplit"):
    """mode: 'actsplit' (2 muls on ACT engine) | 'full' (all-DVE blend) |
    'noblend' (gather+copy+store) | 'nodma' (blend from const tile)."""
    import concourse.bass as bass
    import concourse.bacc as bacc
    import concourse.tile as tile
    from concourse import mybir

    F32 = mybir.dt.float32
    F16 = mybir.dt.float16
    I32 = mybir.dt.int32
    Alu = mybir.AluOpType

    nc = bacc.Bacc()
    fgat = nc.declare_dram_parameter("fgat", [NSLOT, 2 * C], F16, isOutput=False)
    # meta = [w00 | w10 | w01 | w11 | idx-as-f32], one load for the whole head
    meta = nc.declare_dram_parameter("meta", [P, 5 * NPT], F32, isOutput=False)
    out = nc.declare_dram_parameter("out", [P, NPT * C], F16, isOutput=True)

    with tile.TileContext(nc) as tc:
        with (
            tc.tile_pool(name="const", bufs=1) as cpool,
            tc.tile_pool(name="apool", bufs=abufs) as apool,
            tc.tile_pool(name="gpool", bufs=bufs) as wpool,
            tc.tile_pool(name="tpool", bufs=tbufs) as tpool,
        ):
            w = cpool.tile([P, 5 * NPT], F32)
            nc.sync.dma_start(out=w[:], in_=meta[:])
            idx = cpool.tile([P, NPT], I32)
            nc.vector.tensor_copy(out=idx[:], in_=w[:, 4 * NPT:5 * NPT])
            # gathered layout per point: [A0, B0, A1, B1] = [(y0,x0),(y0+1,x0),
            # (y0,x0+1),(y0+1,x0+1)]
            w00 = w[:, 0 * NPT:1 * NPT]
            w10 = w[:, 1 * NPT:2 * NPT]
            w01 = w[:, 2 * NPT:3 * NPT]
            w11 = w[:, 3 * NPT:4 * NPT]

            import concourse.bass as _b

            gconst = None
            if mode == "nodma":
                gconst = cpool.tile([P, 4 * C], F16, tag="gconst")
                nc.vector.memset(gconst[:], 0.25)

            sg = store_group
            assert NPT % sg == 0
            for rep in range(repeats):
                for g_i in range(NPT // sg):
                    afat = apool.tile([P, sg * C], F16, tag="afat")
                    for k in range(sg):
                        t = g_i * sg + k
                        if mode == "nodma":
                            g4 = gconst
                        else:
                            g4 = wpool.tile([P, 4 * C], F16, tag="g4")
                            nc.gpsimd.indirect_dma_start(
                                out=g4[:], out_offset=None, in_=fgat[:],
                                in_offset=_b.IndirectOffsetOnAxis(
                                    ap=idx[:, t:t + 1], axis=0))
                        ac = afat[:, k * C:(k + 1) * C]
                        if mode == "noblend":
                            nc.vector.tensor_copy(out=ac, in_=g4[:, 0:C])
                            continue
                        if mode in ("actsplit", "nodma"):
                            u1 = tpool.tile([P, C], F16, tag="u1")
                            nc.scalar.mul(u1[:], g4[:, C:2 * C], w10[:, t:t + 1])
                            u2 = tpool.tile([P, C], F16, tag="u2")
                            nc.scalar.mul(u2[:], g4[:, 3 * C:4 * C], w11[:, t:t + 1])
                            t1 = tpool.tile([P, C], F16, tag="t1")
                            nc.vector.tensor_scalar(
                                out=t1[:], in0=g4[:, 0:C], scalar1=w00[:, t:t + 1],
                                scalar2=None, op0=Alu.mult)
                            t2 = tpool.tile([P, C], F16, tag="t2")
                            nc.vector.tensor_scalar(
                                out=t2[:], in0=g4[:, 2 * C:3 * C],
                                scalar1=w01[:, t:t + 1],
                                scalar2=None, op0=Alu.mult)
                            nc.vector.tensor_tensor(out=t1[:], in0=t1[:],
                                                    in1=u1[:], op=Alu.add)
                            nc.vector.tensor_tensor(out=t2[:], in0=t2[:],
                                                    in1=u2[:], op=Alu.add)
                            nc.vector.tensor_tensor(out=ac, in0=t1[:],
                                                    in1=t2[:], op=Alu.add)
                            continue
                        nc.vector.tensor_scalar(
                            out=ac, in0=g4[:, 0:C], scalar1=w00[:, t:t + 1],
                            scalar2=None, op0=Alu.mult)
                        nc.vector.scalar_tensor_tensor(
                            out=ac, in0=g4[:, C:2 * C], scalar=w10[:, t:t + 1],
                            in1=ac, op0=Alu.mult, op1=Alu.add)
                        nc.vector.scalar_tensor_tensor(
                            out=ac, in0=g4[:, 2 * C:3 * C], scalar=w01[:, t:t + 1],
                            in1=ac, op0=Alu.mult, op1=Alu.add)
                        nc.vector.scalar_tensor_tensor(
                            out=ac, in0=g4[:, 3 * C:4 * C], scalar=w11[:, t:t + 1],
                            in1=ac, op0=Alu.mult, op1=Alu.add)
                    nc.sync.dma_start(
                        out=out[:, g_i * sg * C:(g_i + 1) * sg * C],
                        in_=afat[:])

    nc.compile()
    return nc


def _prep_fgat(features):
    """Pair-interleaved fp16 gather map: E (even row pairs) then O (odd)."""
    f = np.asarray(features, dtype=np.float32)
    p2 = np.zeros((HP2, WP2, C), dtype=np.float16)
    p2[2:130, 2:130, :] = f.astype(np.float16)
    # E[e, x, r, c] = p2[2e+r, x, c]; O[o, x, r, c] = p2[2o+1+r, x, c]
    e = np.ascontiguousarray(
        p2.reshape(NBLK_E, 2, WP2, C).transpose(0, 2, 1, 3)
    ).reshape(NSLOT_E, 2 * C)
    o = np.ascontiguousarray(
        p2[1:131].reshape(NBLK_O, 2, WP2, C).transpose(0, 2, 1, 3)
    ).reshape(NBLK_O * WP2, 2 * C)
    return np.concatenate([e, o], axis=0)


def _prep_wts_idx(boxes):
    """Per-(box,point) gather slot index and bilinear corner weights.

    Mirrors the reference affine-grid math in float32:
      yf = BY*(0.5*bh-0.5) + (yc-1),  xf = BX*(0.5*bw-0.5) + (xc-1)
    with BY/BX the 7x7 [-1,1] grid; then y0=floor(yf), wy=yf-y0 (same for x).
    OOB corners are mapped to zero border rows/cols of the padded map, so the
    weights need no in-bounds masking.
    """
    b = np.asarray(boxes, dtype=np.float32)
    xc, yc, bw, bh = b[:, 0:1], b[:, 1:2], b[:, 2:3], b[:, 3:4]
    base = np.linspace(-1.0, 1.0, 7).astype(np.float32)
    BY = np.repeat(base, 7)[None, :]   # (1,49)
    BX = np.tile(base, 7)[None, :]
    yf = (BY * (np.float32(0.5) * bh - np.float32(0.5)) + (yc - 1)).astype(np.float32)
    xf = (BX * (np.float32(0.5) * bw - np.float32(0.5)) + (xc - 1)).astype(np.float32)
    y0 = np.floor(yf)
    x0 = np.floor(xf)
    wy = yf - y0
    wx = xf - x0
    wyc = np.float32(1.0) - wy
    wxc = np.float32(1.0) - wx
    # weights for gathered layout [A0, B0, A1, B1]
    wts = np.concatenate([wyc * wxc, wy * wxc, wyc * wx, wy * wx], axis=1)
    # slot = par*NSLOT_E + half*WP2 + clamp(x0,-2,128)+2
    pyA = np.clip(y0 + 2.0, 0.0, 130.0)
    half = np.floor(pyA * 0.5)
    par = pyA - 2.0 * half
    px = np.clip(x0, -2.0, 128.0) + 2.0
    slot = par * NSLOT_E + half * WP2 + px
    # slot values < 2^24, exactly representable in f32: ship as f32, cast
    # to int32 on device (one tensor_copy) so the head is a single DMA load
    meta = np.concatenate([wts, slot], axis=1)
    return np.ascontiguousarray(meta, dtype=np.float32)


def _in_maps(features, boxes):
    fgat = _prep_fgat(features)
    meta = _prep_wts_idx(boxes)
    return [
        {
            "fgat": fgat,
            "meta": np.ascontiguousarray(meta[k * P:(k + 1) * P]),
        }
        for k in range(NCORES)
    ]


def kernel(features, boxes, image_height=128, image_width=128):
    from concourse.bass_utils import run_bass_kernel_spmd

    if "nc" not in _STATE:
        _STATE["nc"] = _build_nc()
    nc = _STATE["nc"]

    in_maps = _in_maps(features, boxes)
    res = run_bass_kernel_spmd(
        nc, in_maps, core_ids=list(range(NCORES)),
        trace=_STATE.get("trace", False),
    )
    _STATE["last"] = res
    out = np.concatenate(
        [res.results[k]["out"].reshape(P, 7, 7, C).astype(np.float32)
         for k in range(NCORES)],
        axis=0,
    )
    return out
